# revision 6
# baseline (speedup 1.0000x reference)
"""Multi-head self-attention (B=4, C=256, H=W=48, NH=8) on 8 TRN2 NeuronCores.

Sharding: 8 shards = 4 batches x 2 query-halves (no collectives).
Each core: K,V projections for its batch over all S=2304 positions,
Q projection for its 1152-query half, attention for all 8 heads over
its query half, output projection + bias + residual for its disjoint
[256, 1152] output slice.

Kernel layout notes:
  - All matmuls run as float32r (full PE rate at free-dim >= 256). The
    BIR verifier requires f32r matmul inputs to be *produced rounded*
    by a compute op, so Q/K/V^T/exp tiles are written as f32r by their
    producing DVE/ACT ops, and x / weights get explicit DVE rounding
    copies after the DMA load.
  - Scores are computed transposed, [t, q], so the A@V matmul needs no
    transposes anywhere: lhsT = K[d, t] slice, rhs = Q[d, q] slice.
  - exp runs on ScalarE directly from PSUM into SBUF (the PSUM->SBUF
    move is fused into the activation); softmax max-subtraction is
    skipped (scores ~ N(0,1), no overflow risk in f32).
  - The softmax denominator comes free from the A@V matmul via a ones
    column appended to V^T (lhsT is [128, 33]; row 32 of the psum
    accumulates sum_t exp(s)).
  - Normalization: DVE reciprocal of the denom row (lane 32), bounce
    through a DRAM scratch row, DMA-broadcast it across 32 partitions
    (stride-0 partition APs are only legal from DRAM), DVE multiply
    (one PSUM operand only), then a DMA moves the result to its head
    slot in attnout (DVE cannot shift partitions; DMA can).
  - This container's walrus allows ONE sem wait per instruction; a
    post-scheduling pass splits multi-wait instructions into
    single-wait same-engine nops (see _TileContextP).
"""

import numpy as np

import concourse.bass as bass
import concourse.mybir as mybir
import concourse.tile as tile
from concourse.vector_clock import ScopedClock
from concourse.bass_utils import run_bass_kernel_spmd

B, C, HH, WW = 4, 256, 48, 48
S = HH * WW            # 2304
NH, HD = 8, 32
SCALE = HD ** -0.5
SQ = S // 2            # 1152 queries per core
QC = 384               # q-chunk (fits one PSUM bank in f32)
NQC = SQ // QC         # 3
NTT = S // 128         # 18 t-tiles
TG = 3                 # t-tiles per exp group
NTG = NTT // TG        # 6
CT = C // 128          # 2 channel tiles

F32 = mybir.dt.float32
F32R = mybir.dt.float32r
AF = mybir.ActivationFunctionType
ALU = mybir.AluOpType

N_CORES = 8


class _TileContextP(tile.TileContext):
    """TileContext adapted to a walrus that allows 1 sem wait/instruction.

    After Tile scheduling, every instruction carrying N>1 sem waits is
    rewritten to keep its last wait; the other N-1 waits move onto
    fresh single-wait nops inserted just before it on the same engine
    (engines execute their stream in order, so blocking at the nop is
    equivalent). The kernel-tail drain is built the same way.
    """

    def _split_multi_waits(self):
        nc = self.nc
        for fn in nc.m.functions:
            for bb in fn.blocks:
                new_insts = []
                for inst in bb.instructions:
                    si = inst.sync_info
                    if si is not None and len(si.on_wait) > 1:
                        waits = list(si.on_wait)
                        for w in waits[:-1]:
                            nop = mybir.InstNoOp(
                                name=nc.get_next_instruction_name(),
                                engine=inst.engine,
                                ins=[], outs=[],
                                sync_info=mybir.SyncInfo(on_wait=[w], on_update=[]),
                                bass_nofuse=True,
                            )
                            nc.register_instruction(nop, overwrite=True)
                            new_insts.append(nop)
                        inst.sync_info = mybir.SyncInfo(
                            on_wait=[waits[-1]], on_update=list(si.on_update)
                        )
                    new_insts.append(inst)
                bb.instructions = new_insts

    def _drain_and_barrier(self, tick_clock, wait_clock):
        carrier = self.nc.sync.nop(nofuse=True)
        wait_clock.add_sem_waits(
            carrier.ins, ScopedClock({None: tick_clock.global_clock})
        )
        self.nc.sync.drain()
        self.nc.all_engine_barrier()
        assert self.sems is not None
        popped = self.nc._tile_sem_poison_stack.pop()
        assert popped is self._sem_poison
        self.nc.clear_and_free_semaphores(list(self.sems.allocated().values()))
        self.nc.all_engine_barrier()
        self._split_multi_waits()


def _build_nc():
    nc = bass.Bass()

    xf_d = nc.dram_tensor("xf", [C, S], F32, kind="ExternalInput")
    xq_d = nc.dram_tensor("xq", [C, SQ], F32, kind="ExternalInput")
    wqt_d = nc.dram_tensor("wqt", [C, C], F32, kind="ExternalInput")
    wkt_d = nc.dram_tensor("wkt", [C, C], F32, kind="ExternalInput")
    wvt_d = nc.dram_tensor("wvt", [C, C], F32, kind="ExternalInput")
    wot_d = nc.dram_tensor("wot", [C, C], F32, kind="ExternalInput")
    bqp_d = nc.dram_tensor("bqp", [128, CT], F32, kind="ExternalInput")
    bkp_d = nc.dram_tensor("bkp", [128, CT], F32, kind="ExternalInput")
    bop_d = nc.dram_tensor("bop", [128, CT], F32, kind="ExternalInput")
    bv_d = nc.dram_tensor("bv", [C], F32, kind="ExternalInput")
    out_d = nc.dram_tensor("out", [C, SQ], F32, kind="ExternalOutput")

    with _TileContextP(nc) as tc:
        with (
            tc.tile_pool(name="singles", bufs=1) as singles,
            tc.tile_pool(name="sbig", bufs=1) as sbig,
            tc.tile_pool(name="expsp", bufs=3) as expsp,
            tc.tile_pool(name="smallp", bufs=4) as smallp,
            tc.tile_pool(name="outp", bufs=3) as outp,
            tc.tile_pool(name="drp", bufs=4, space="DRAM") as drp,
        ):
            # ---- static loads + f32r rounding copies --------------------
            w_ld = {}
            w_rb = {}
            for nm, d in (("wqt", wqt_d), ("wkt", wkt_d), ("wvt", wvt_d),
                          ("wot", wot_d)):
                ld = singles.tile([128, CT, C], F32, tag=f"{nm}_ld")
                nc.sync.dma_start(out=ld, in_=d.rearrange("(t p) o -> p t o", p=128))
                rb = singles.tile([128, CT, C], F32R, tag=f"{nm}_rb")
                nc.vector.tensor_copy(out=rb, in_=ld)
                w_ld[nm] = ld
                w_rb[nm] = rb
            wqt_sb, wkt_sb, wvt_sb, wot_sb = (
                w_rb["wqt"], w_rb["wkt"], w_rb["wvt"], w_rb["wot"])

            bqp_sb = singles.tile([128, CT], F32)
            bkp_sb = singles.tile([128, CT], F32)
            bop_sb = singles.tile([128, CT], F32)
            nc.sync.dma_start(out=bqp_sb, in_=bqp_d[:, :])
            nc.sync.dma_start(out=bkp_sb, in_=bkp_d[:, :])
            nc.sync.dma_start(out=bop_sb, in_=bop_d[:, :])

            bv_sb = singles.tile([128, C], F32)
            bv_ap = bv_d[:]
            nc.gpsimd.dma_start(
                out=bv_sb,
                in_=bass.AP(
                    tensor=bv_ap.tensor, offset=bv_ap.offset,
                    ap=[[0, 128]] + [list(a) for a in bv_ap.ap],
                ),
            )

            x_ld = sbig.tile([128, CT, S], F32)
            nc.sync.dma_start(out=x_ld, in_=xf_d.rearrange("(t p) s -> p t s", p=128))
            x_rb = sbig.tile([128, CT, S], F32R)
            nc.vector.tensor_copy(out=x_rb, in_=x_ld)
            xq_ld = sbig.tile([128, CT, SQ], F32)
            nc.sync.dma_start(out=xq_ld, in_=xq_d.rearrange("(t p) s -> p t s", p=128))
            xq_rb = sbig.tile([128, CT, SQ], F32R)
            nc.vector.tensor_copy(out=xq_rb, in_=xq_ld)

            k_sb = sbig.tile([128, CT, S], F32R)
            q_sb = sbig.tile([128, CT, SQ], F32R)
            vt_sb = sbig.tile([128, NTT, NH, HD + 1], F32R)
            att_sb = sbig.tile([128, CT, SQ], F32R)

            ones_f32 = singles.tile([128, NTT, NH], F32)
            nc.vector.memset(ones_f32, 1.0)
            nc.vector.tensor_copy(out=vt_sb[:, :, :, HD], in_=ones_f32)

            # ---- phase A: projections -----------------------------------
            with tc.tile_pool(name="psA", bufs=4, space="PSUM") as psA:
                # Q = WqT.T @ xq  (+bq), laid out [c, q]
                for ot in range(CT):
                    for j in range(NQC):
                        ps = psA.tile([128, QC], F32, tag="proj")
                        for kt in range(CT):
                            nc.tensor.matmul(
                                ps,
                                lhsT=wqt_sb[:, kt, ot * 128:(ot + 1) * 128],
                                rhs=xq_rb[:, kt, j * QC:(j + 1) * QC],
                                start=(kt == 0), stop=(kt == CT - 1),
                            )
                        nc.vector.tensor_scalar(
                            out=q_sb[:, ot, j * QC:(j + 1) * QC],
                            in0=ps, scalar1=bqp_sb[:, ot:ot + 1], scalar2=None,
                            op0=ALU.add,
                        )
                # K = WkT.T @ xf  (+bk), laid out [c, t]
                for ot in range(CT):
                    for j in range(S // QC):
                        ps = psA.tile([128, QC], F32, tag="proj")
                        for kt in range(CT):
                            nc.tensor.matmul(
                                ps,
                                lhsT=wkt_sb[:, kt, ot * 128:(ot + 1) * 128],
                                rhs=x_rb[:, kt, j * QC:(j + 1) * QC],
                                start=(kt == 0), stop=(kt == CT - 1),
                            )
                        nc.vector.tensor_scalar(
                            out=k_sb[:, ot, j * QC:(j + 1) * QC],
                            in0=ps, scalar1=bkp_sb[:, ot:ot + 1], scalar2=None,
                            op0=ALU.add,
                        )
                # V^T = (x slice).T @ WvT  (+bv), laid out [t, c] per tile
                for st in range(NTT):
                    ps = psA.tile([128, C], F32, tag="projv")
                    for kt in range(CT):
                        nc.tensor.matmul(
                            ps,
                            lhsT=x_rb[:, kt, st * 128:(st + 1) * 128],
                            rhs=wvt_sb[:, kt, :],
                            start=(kt == 0), stop=(kt == CT - 1),
                        )
                    nc.vector.tensor_tensor(
                        out=vt_sb[:, st, :, 0:HD],
                        in0=ps.rearrange("p (h d) -> p h d", d=HD),
                        in1=bv_sb.rearrange("p (h d) -> p h d", d=HD),
                        op=ALU.add,
                    )

            # ---- phase B: attention -------------------------------------
            with (
                tc.tile_pool(name="scp", bufs=2, space="PSUM") as scp,
                tc.tile_pool(name="avp", bufs=2, space="PSUM") as avp,
            ):
                for h in range(NH):
                    ct = h // 4
                    co = 32 * (h % 4)
                    for j in range(NQC):
                        av = avp.tile([HD + 1, QC], F32, tag="av")
                        for g in range(NTG):
                            sc = scp.tile([128, TG, 512], F32, tag="sc")
                            for tt in range(TG):
                                t0 = (g * TG + tt) * 128
                                nc.tensor.matmul(
                                    sc[:, tt, 0:QC],
                                    lhsT=k_sb[co:co + HD, ct, t0:t0 + 128],
                                    rhs=q_sb[co:co + HD, ct, j * QC:(j + 1) * QC],
                                    start=True, stop=True,
                                    tile_position=(co, 0),
                                )
                            ex = expsp.tile([128, TG, QC], F32R, tag="ex")
                            nc.scalar.activation(
                                out=ex, in_=sc[:, :, 0:QC], func=AF.Exp, scale=SCALE,
                            )
                            for tt in range(TG):
                                nc.tensor.matmul(
                                    av,
                                    lhsT=vt_sb[:, g * TG + tt, h, :],
                                    rhs=ex[:, tt, :],
                                    start=(g == 0 and tt == 0),
                                    stop=(g == NTG - 1 and tt == TG - 1),
                                )
                        # normalize: recip on lane 32, DRAM-bounce broadcast,
                        # lane-aligned multiply, DMA shift into att_sb.
                        rec = smallp.tile([HD + 1, QC], F32, tag="rec")
                        nc.vector.reciprocal(rec[HD:HD + 1, :], av[HD:HD + 1, :])
                        dscr = drp.tile([QC], F32, tag="dscr")
                        nc.sync.dma_start(out=dscr[:], in_=rec[HD:HD + 1, :])
                        bc = smallp.tile([HD, QC], F32, tag="bc")
                        dap = dscr[:]
                        nc.gpsimd.dma_start(
                            out=bc,
                            in_=bass.AP(
                                tensor=dap.tensor, offset=dap.offset,
                                ap=[[0, HD]] + [list(a) for a in dap.ap],
                            ),
                        )
                        nrm = smallp.tile([HD, QC], F32R, tag="nrm")
                        nc.vector.tensor_tensor(
                            out=nrm, in0=av[0:HD, :], in1=bc, op=ALU.mult,
                        )
                        nc.sync.dma_start(
                            out=att_sb[co:co + HD, ct, j * QC:(j + 1) * QC],
                            in_=nrm,
                        )

            # ---- phase C: output projection + bias + residual -----------
            out_r = out_d.rearrange("(t p) q -> p t q", p=128)
            with tc.tile_pool(name="psC", bufs=2, space="PSUM") as psC:
                for ot in range(CT):
                    for j in range(NQC):
                        ps = psC.tile([128, QC], F32, tag="proj")
                        for kt in range(CT):
                            nc.tensor.matmul(
                                ps,
                                lhsT=wot_sb[:, kt, ot * 128:(ot + 1) * 128],
                                rhs=att_sb[:, kt, j * QC:(j + 1) * QC],
                                start=(kt == 0), stop=(kt == CT - 1),
                            )
                        ob = outp.tile([128, QC], F32, tag="ob")
                        nc.vector.tensor_scalar(
                            out=ob, in0=ps, scalar1=bop_sb[:, ot:ot + 1],
                            scalar2=None, op0=ALU.add,
                        )
                        nc.vector.tensor_tensor(
                            out=ob, in0=ob,
                            in1=xq_ld[:, ot, j * QC:(j + 1) * QC], op=ALU.add,
                        )
                        nc.sync.dma_start(
                            out=out_r[:, ot, j * QC:(j + 1) * QC], in_=ob,
                        )

    return nc


_NC = None
LAST_RESULTS = None
TRACE = False


def _get_nc():
    global _NC
    if _NC is None:
        _NC = _build_nc()
    return _NC


def kernel(x, Wq, bq, Wk, bk, Wv, bv, Wo, bo):
    global LAST_RESULTS
    x = np.ascontiguousarray(np.asarray(x, dtype=np.float32).reshape(B, C, S))
    wqt = np.ascontiguousarray(np.asarray(Wq, dtype=np.float32).T)
    wkt = np.ascontiguousarray(np.asarray(Wk, dtype=np.float32).T)
    wvt = np.ascontiguousarray(np.asarray(Wv, dtype=np.float32).T)
    wot = np.ascontiguousarray(np.asarray(Wo, dtype=np.float32).T)
    bqp = np.ascontiguousarray(np.asarray(bq, dtype=np.float32).reshape(CT, 128).T)
    bkp = np.ascontiguousarray(np.asarray(bk, dtype=np.float32).reshape(CT, 128).T)
    bop = np.ascontiguousarray(np.asarray(bo, dtype=np.float32).reshape(CT, 128).T)
    bvv = np.ascontiguousarray(np.asarray(bv, dtype=np.float32))

    in_maps = []
    for core in range(N_CORES):
        b, half = divmod(core, 2)
        qlo = half * SQ
        in_maps.append({
            "xf": x[b],
            "xq": np.ascontiguousarray(x[b][:, qlo:qlo + SQ]),
            "wqt": wqt, "wkt": wkt, "wvt": wvt, "wot": wot,
            "bqp": bqp, "bkp": bkp, "bop": bop, "bv": bvv,
        })

    res = run_bass_kernel_spmd(_get_nc(), in_maps, list(range(N_CORES)), trace=TRACE)
    LAST_RESULTS = res

    out = np.empty((B, C, S), dtype=np.float32)
    for core in range(N_CORES):
        b, half = divmod(core, 2)
        qlo = half * SQ
        out[b][:, qlo:qlo + SQ] = res.results[core]["out"]
    return out.reshape(B, C, HH, WW)


# revision 9
# speedup vs baseline: 1.1333x; 1.1333x over previous
"""Multi-head self-attention (B=4, C=256, H=W=48, NH=8) on 8 TRN2 NeuronCores.

Sharding: 8 shards = 4 batches x 2 query-halves (no collectives).
Each core: K,V projections for its batch over all S=2304 positions,
Q projection for its 1152-query half, attention for all 8 heads over
its query half, output projection + bias + residual for its disjoint
[256, 1152] output slice.

Kernel layout notes:
  - All matmuls run as float32r (full PE rate at free-dim >= 256). The
    BIR verifier requires f32r matmul inputs to be *produced rounded*
    by a compute op, so Q/K/V^T/exp tiles are written as f32r by their
    producing DVE/ACT ops, and x / weights get explicit DVE rounding
    copies after the DMA load.
  - Scores are computed transposed, [t, q], so the A@V matmul needs no
    transposes anywhere: lhsT = K[d, t] slice, rhs = Q[d, q] slice.
  - exp runs on ScalarE directly from PSUM into SBUF (the PSUM->SBUF
    move is fused into the activation); softmax max-subtraction is
    skipped (scores ~ N(0,1), no overflow risk in f32).
  - The softmax denominator comes free from the A@V matmul via a ones
    column appended to V^T (lhsT is [128, 33]; row 32 of the psum
    accumulates sum_t exp(s)).
  - Normalization: DVE reciprocal of the denom row (lane 32), bounce
    through a DRAM scratch row, DMA-broadcast it across 32 partitions
    (stride-0 partition APs are only legal from DRAM), DVE multiply
    (one PSUM operand only), then a DMA moves the result to its head
    slot in attnout (DVE cannot shift partitions; DMA can).
  - This container's walrus allows ONE sem wait per instruction; a
    post-scheduling pass splits multi-wait instructions into
    single-wait same-engine nops (see _TileContextP).
"""

import numpy as np

import concourse.bass as bass
import concourse.mybir as mybir
import concourse.tile as tile
from concourse.vector_clock import ScopedClock
from concourse.bass_utils import run_bass_kernel_spmd

B, C, HH, WW = 4, 256, 48, 48
S = HH * WW            # 2304
NH, HD = 8, 32
SCALE = HD ** -0.5
SQ = S // 2            # 1152 queries per core
QC = 384               # q-chunk (fits one PSUM bank in f32)
NQC = SQ // QC         # 3
NTT = S // 128         # 18 t-tiles
TG = 3                 # t-tiles per exp group
NTG = NTT // TG        # 6
CT = C // 128          # 2 channel tiles

F32 = mybir.dt.float32
F32R = mybir.dt.float32r
BF16 = mybir.dt.bfloat16
AF = mybir.ActivationFunctionType
ALU = mybir.AluOpType

N_CORES = 8


class _TileContextP(tile.TileContext):
    """TileContext adapted to a walrus that allows 1 sem wait/instruction.

    After Tile scheduling, every instruction carrying N>1 sem waits is
    rewritten to keep its last wait; the other N-1 waits move onto
    fresh single-wait nops inserted just before it on the same engine
    (engines execute their stream in order, so blocking at the nop is
    equivalent). The kernel-tail drain is built the same way.
    """

    def _split_multi_waits(self):
        nc = self.nc
        for fn in nc.m.functions:
            for bb in fn.blocks:
                new_insts = []
                for inst in bb.instructions:
                    si = inst.sync_info
                    if si is not None and len(si.on_wait) > 1:
                        waits = list(si.on_wait)
                        for w in waits[:-1]:
                            nop = mybir.InstNoOp(
                                name=nc.get_next_instruction_name(),
                                engine=inst.engine,
                                ins=[], outs=[],
                                sync_info=mybir.SyncInfo(on_wait=[w], on_update=[]),
                                bass_nofuse=True,
                            )
                            nc.register_instruction(nop, overwrite=True)
                            new_insts.append(nop)
                        inst.sync_info = mybir.SyncInfo(
                            on_wait=[waits[-1]], on_update=list(si.on_update)
                        )
                    new_insts.append(inst)
                bb.instructions = new_insts

    def _drain_and_barrier(self, tick_clock, wait_clock):
        carrier = self.nc.sync.nop(nofuse=True)
        wait_clock.add_sem_waits(
            carrier.ins, ScopedClock({None: tick_clock.global_clock})
        )
        self.nc.sync.drain()
        self.nc.all_engine_barrier()
        assert self.sems is not None
        popped = self.nc._tile_sem_poison_stack.pop()
        assert popped is self._sem_poison
        self.nc.clear_and_free_semaphores(list(self.sems.allocated().values()))
        self.nc.all_engine_barrier()
        self._split_multi_waits()


def _build_nc():
    nc = bass.Bass()

    xf_d = nc.dram_tensor("xf", [C, S], F32, kind="ExternalInput")
    xq_d = nc.dram_tensor("xq", [C, SQ], F32, kind="ExternalInput")
    wqt_d = nc.dram_tensor("wqt", [C, C], F32, kind="ExternalInput")
    wkt_d = nc.dram_tensor("wkt", [C, C], F32, kind="ExternalInput")
    wvt_d = nc.dram_tensor("wvt", [C, C], F32, kind="ExternalInput")
    wot_d = nc.dram_tensor("wot", [C, C], F32, kind="ExternalInput")
    bqp_d = nc.dram_tensor("bqp", [128, CT], F32, kind="ExternalInput")
    bkp_d = nc.dram_tensor("bkp", [128, CT], F32, kind="ExternalInput")
    bop_d = nc.dram_tensor("bop", [128, CT], F32, kind="ExternalInput")
    bv_d = nc.dram_tensor("bv", [C], F32, kind="ExternalInput")
    out_d = nc.dram_tensor("out", [C, SQ], F32, kind="ExternalOutput")

    with _TileContextP(nc) as tc:
        with (
            tc.tile_pool(name="singles", bufs=1) as singles,
            tc.tile_pool(name="sbig", bufs=1) as sbig,
            tc.tile_pool(name="expsp", bufs=3) as expsp,
            tc.tile_pool(name="smallp", bufs=4) as smallp,
            tc.tile_pool(name="outp", bufs=3) as outp,
            tc.tile_pool(name="drp", bufs=4, space="DRAM") as drp,
        ):
            # ---- static loads + f32r rounding copies --------------------
            w_ld = {}
            w_rb = {}
            for nm, d in (("wqt", wqt_d), ("wkt", wkt_d), ("wvt", wvt_d),
                          ("wot", wot_d)):
                ld = singles.tile([128, CT, C], F32, tag=f"{nm}_ld")
                nc.sync.dma_start(out=ld, in_=d.rearrange("(t p) o -> p t o", p=128))
                rb = singles.tile([128, CT, C], F32R, tag=f"{nm}_rb")
                nc.vector.tensor_copy(out=rb, in_=ld)
                w_ld[nm] = ld
                w_rb[nm] = rb
            wqt_sb, wkt_sb, wvt_sb, wot_sb = (
                w_rb["wqt"], w_rb["wkt"], w_rb["wvt"], w_rb["wot"])

            bqp_sb = singles.tile([128, CT], F32)
            bkp_sb = singles.tile([128, CT], F32)
            bop_sb = singles.tile([128, CT], F32)
            nc.sync.dma_start(out=bqp_sb, in_=bqp_d[:, :])
            nc.sync.dma_start(out=bkp_sb, in_=bkp_d[:, :])
            nc.sync.dma_start(out=bop_sb, in_=bop_d[:, :])

            bv_sb = singles.tile([128, C], F32)
            bv_ap = bv_d[:]
            nc.gpsimd.dma_start(
                out=bv_sb,
                in_=bass.AP(
                    tensor=bv_ap.tensor, offset=bv_ap.offset,
                    ap=[[0, 128]] + [list(a) for a in bv_ap.ap],
                ),
            )

            x_ld = sbig.tile([128, CT, S], F32)
            nc.sync.dma_start(out=x_ld, in_=xf_d.rearrange("(t p) s -> p t s", p=128))
            x_rb = sbig.tile([128, CT, S], F32R)
            nc.vector.tensor_copy(out=x_rb, in_=x_ld)
            xq_ld = sbig.tile([128, CT, SQ], F32)
            nc.sync.dma_start(out=xq_ld, in_=xq_d.rearrange("(t p) s -> p t s", p=128))
            xq_rb = sbig.tile([128, CT, SQ], F32R)
            nc.vector.tensor_copy(out=xq_rb, in_=xq_ld)

            k_sb = sbig.tile([128, CT, S], F32R)
            q_sb = sbig.tile([128, CT, SQ], F32R)
            vt_sb = sbig.tile([128, NTT, NH, HD + 1], BF16)
            att_sb = sbig.tile([128, CT, SQ], F32R)

            ones_f32 = singles.tile([128, NTT, NH], F32)
            nc.vector.memset(ones_f32, 1.0)
            nc.vector.tensor_copy(out=vt_sb[:, :, :, HD], in_=ones_f32)

            # ---- phase A: projections -----------------------------------
            with tc.tile_pool(name="psA", bufs=4, space="PSUM") as psA:
                # Q = WqT.T @ xq  (+bq), laid out [c, q]
                for ot in range(CT):
                    for j in range(NQC):
                        ps = psA.tile([128, QC], F32, tag="proj")
                        for kt in range(CT):
                            nc.tensor.matmul(
                                ps,
                                lhsT=wqt_sb[:, kt, ot * 128:(ot + 1) * 128],
                                rhs=xq_rb[:, kt, j * QC:(j + 1) * QC],
                                start=(kt == 0), stop=(kt == CT - 1),
                            )
                        nc.vector.tensor_scalar(
                            out=q_sb[:, ot, j * QC:(j + 1) * QC],
                            in0=ps, scalar1=bqp_sb[:, ot:ot + 1], scalar2=None,
                            op0=ALU.add,
                        )
                # K = WkT.T @ xf  (+bk), laid out [c, t]
                for ot in range(CT):
                    for j in range(S // QC):
                        ps = psA.tile([128, QC], F32, tag="proj")
                        for kt in range(CT):
                            nc.tensor.matmul(
                                ps,
                                lhsT=wkt_sb[:, kt, ot * 128:(ot + 1) * 128],
                                rhs=x_rb[:, kt, j * QC:(j + 1) * QC],
                                start=(kt == 0), stop=(kt == CT - 1),
                            )
                        nc.vector.tensor_scalar(
                            out=k_sb[:, ot, j * QC:(j + 1) * QC],
                            in0=ps, scalar1=bkp_sb[:, ot:ot + 1], scalar2=None,
                            op0=ALU.add,
                        )
                # V^T = (x slice).T @ WvT  (+bv), laid out [t, c] per tile
                for st in range(NTT):
                    ps = psA.tile([128, C], F32, tag="projv")
                    for kt in range(CT):
                        nc.tensor.matmul(
                            ps,
                            lhsT=x_rb[:, kt, st * 128:(st + 1) * 128],
                            rhs=wvt_sb[:, kt, :],
                            start=(kt == 0), stop=(kt == CT - 1),
                        )
                    nc.vector.tensor_tensor(
                        out=vt_sb[:, st, :, 0:HD],
                        in0=ps.rearrange("p (h d) -> p h d", d=HD),
                        in1=bv_sb.rearrange("p (h d) -> p h d", d=HD),
                        op=ALU.add,
                    )

            # ---- phase B: attention -------------------------------------
            # Heads run in pairs (2i, 2i+1): their scores matmuls sit in
            # distinct 32-row groups of the PE array (K=32 contraction) and
            # their A@V matmuls in distinct column strips (head a -> psum
            # rows 0:33, head b -> rows 64:97 of ONE bank), so each pair's
            # matmuls execute concurrently in the array.
            with (
                tc.tile_pool(name="scp", bufs=2, space="PSUM") as scp,
                tc.tile_pool(name="avp", bufs=2, space="PSUM") as avp,
            ):
                for hp in range(NH // 2):
                    ha, hb = 2 * hp, 2 * hp + 1
                    cta, coa = ha // 4, 32 * (ha % 4)
                    ctb, cob = hb // 4, 32 * (hb % 4)
                    for j in range(NQC):
                        js = slice(j * QC, (j + 1) * QC)
                        av = avp.tile([97, QC], F32, tag="av")
                        for g in range(NTG):
                            sca = scp.tile([128, TG, 512], F32, tag="sc")
                            scb = scp.tile([128, TG, 512], F32, tag="sc")
                            for tt in range(TG):
                                t0 = (g * TG + tt) * 128
                                nc.tensor.matmul(
                                    sca[:, tt, 0:QC],
                                    lhsT=k_sb[coa:coa + HD, cta, t0:t0 + 128],
                                    rhs=q_sb[coa:coa + HD, cta, js],
                                    start=True, stop=True,
                                    tile_position=(coa, 0),
                                )
                                nc.tensor.matmul(
                                    scb[:, tt, 0:QC],
                                    lhsT=k_sb[cob:cob + HD, ctb, t0:t0 + 128],
                                    rhs=q_sb[cob:cob + HD, ctb, js],
                                    start=True, stop=True,
                                    tile_position=(cob, 0),
                                )
                            exa = expsp.tile([128, TG, QC], BF16, tag="ex")
                            nc.scalar.activation(
                                out=exa, in_=sca[:, :, 0:QC], func=AF.Exp, scale=SCALE,
                            )
                            exb = expsp.tile([128, TG, QC], BF16, tag="ex")
                            nc.scalar.activation(
                                out=exb, in_=scb[:, :, 0:QC], func=AF.Exp, scale=SCALE,
                            )
                            for tt in range(TG):
                                st = g * TG + tt
                                first = (g == 0 and tt == 0)
                                last = (g == NTG - 1 and tt == TG - 1)
                                nc.tensor.matmul(
                                    av[0:HD + 1, :],
                                    lhsT=vt_sb[:, st, ha, :],
                                    rhs=exa[:, tt, :],
                                    start=first, stop=last,
                                    tile_position=(0, 0),
                                    skip_group_check=True,
                                )
                                nc.tensor.matmul(
                                    av[64:64 + HD + 1, :],
                                    lhsT=vt_sb[:, st, hb, :],
                                    rhs=exb[:, tt, :],
                                    start=first, stop=last,
                                    tile_position=(0, 64),
                                    skip_group_check=True,
                                )
                        # normalize both heads: one reciprocal over the two
                        # denom lanes (32, 96), DRAM-bounce broadcast per
                        # head, lane-aligned multiplies, DMA shift into
                        # att_sb head slots.
                        rec = smallp.tile([97, QC], F32, tag="rec")
                        nc.vector.reciprocal(rec[HD:HD + 1, :], av[HD:HD + 1, :])
                        nc.vector.reciprocal(rec[96:97, :], av[96:97, :])
                        dscr = drp.tile([2, QC], F32, tag="dscr")
                        nc.sync.dma_start(out=dscr[0:1, :], in_=rec[HD:HD + 1, :])
                        nc.sync.dma_start(out=dscr[1:2, :], in_=rec[96:97, :])
                        bc = smallp.tile([96, QC], F32, tag="bc")
                        for hi in range(2):
                            dap = dscr[hi:hi + 1, :]
                            nc.gpsimd.dma_start(
                                out=bc[64 * hi:64 * hi + HD, :],
                                in_=bass.AP(
                                    tensor=dap.tensor, offset=dap.offset,
                                    ap=[[0, HD]] + [list(a) for a in dap.ap[1:]],
                                ),
                            )
                        nrm = smallp.tile([96, QC], F32R, tag="nrm")
                        nc.vector.tensor_tensor(
                            out=nrm[0:HD, :], in0=av[0:HD, :],
                            in1=bc[0:HD, :], op=ALU.mult,
                        )
                        nc.vector.tensor_tensor(
                            out=nrm[64:64 + HD, :], in0=av[64:64 + HD, :],
                            in1=bc[64:64 + HD, :], op=ALU.mult,
                        )
                        nc.sync.dma_start(
                            out=att_sb[coa:coa + HD, cta, js], in_=nrm[0:HD, :],
                        )
                        nc.sync.dma_start(
                            out=att_sb[cob:cob + HD, ctb, js],
                            in_=nrm[64:64 + HD, :],
                        )

            # ---- phase C: output projection + bias + residual -----------
            out_r = out_d.rearrange("(t p) q -> p t q", p=128)
            with tc.tile_pool(name="psC", bufs=2, space="PSUM") as psC:
                for ot in range(CT):
                    for j in range(NQC):
                        ps = psC.tile([128, QC], F32, tag="proj")
                        for kt in range(CT):
                            nc.tensor.matmul(
                                ps,
                                lhsT=wot_sb[:, kt, ot * 128:(ot + 1) * 128],
                                rhs=att_sb[:, kt, j * QC:(j + 1) * QC],
                                start=(kt == 0), stop=(kt == CT - 1),
                            )
                        ob = outp.tile([128, QC], F32, tag="ob")
                        nc.vector.tensor_scalar(
                            out=ob, in0=ps, scalar1=bop_sb[:, ot:ot + 1],
                            scalar2=None, op0=ALU.add,
                        )
                        nc.vector.tensor_tensor(
                            out=ob, in0=ob,
                            in1=xq_ld[:, ot, j * QC:(j + 1) * QC], op=ALU.add,
                        )
                        nc.sync.dma_start(
                            out=out_r[:, ot, j * QC:(j + 1) * QC], in_=ob,
                        )

    return nc


_NC = None
LAST_RESULTS = None
TRACE = False


def _get_nc():
    global _NC
    if _NC is None:
        _NC = _build_nc()
    return _NC


def kernel(x, Wq, bq, Wk, bk, Wv, bv, Wo, bo):
    global LAST_RESULTS
    x = np.ascontiguousarray(np.asarray(x, dtype=np.float32).reshape(B, C, S))
    wqt = np.ascontiguousarray(np.asarray(Wq, dtype=np.float32).T)
    wkt = np.ascontiguousarray(np.asarray(Wk, dtype=np.float32).T)
    wvt = np.ascontiguousarray(np.asarray(Wv, dtype=np.float32).T)
    wot = np.ascontiguousarray(np.asarray(Wo, dtype=np.float32).T)
    bqp = np.ascontiguousarray(np.asarray(bq, dtype=np.float32).reshape(CT, 128).T)
    bkp = np.ascontiguousarray(np.asarray(bk, dtype=np.float32).reshape(CT, 128).T)
    bop = np.ascontiguousarray(np.asarray(bo, dtype=np.float32).reshape(CT, 128).T)
    bvv = np.ascontiguousarray(np.asarray(bv, dtype=np.float32))

    in_maps = []
    for core in range(N_CORES):
        b, half = divmod(core, 2)
        qlo = half * SQ
        in_maps.append({
            "xf": x[b],
            "xq": np.ascontiguousarray(x[b][:, qlo:qlo + SQ]),
            "wqt": wqt, "wkt": wkt, "wvt": wvt, "wot": wot,
            "bqp": bqp, "bkp": bkp, "bop": bop, "bv": bvv,
        })

    res = run_bass_kernel_spmd(_get_nc(), in_maps, list(range(N_CORES)), trace=TRACE)
    LAST_RESULTS = res

    out = np.empty((B, C, S), dtype=np.float32)
    for core in range(N_CORES):
        b, half = divmod(core, 2)
        qlo = half * SQ
        out[b][:, qlo:qlo + SQ] = res.results[core]["out"]
    return out.reshape(B, C, HH, WW)


# revision 10
# speedup vs baseline: 1.3166x; 1.1617x over previous
"""Multi-head self-attention (B=4, C=256, H=W=48, NH=8) on 8 TRN2 NeuronCores.

Sharding: 8 shards = 4 batches x 2 query-halves (no collectives).
Each core: K,V projections for its batch over all S=2304 positions,
Q projection for its 1152-query half, attention for all 8 heads over
its query half, output projection + bias + residual for its disjoint
[256, 1152] output slice.

Kernel layout notes:
  - All matmuls run as float32r (full PE rate at free-dim >= 256). The
    BIR verifier requires f32r matmul inputs to be *produced rounded*
    by a compute op, so Q/K/V^T/exp tiles are written as f32r by their
    producing DVE/ACT ops, and x / weights get explicit DVE rounding
    copies after the DMA load.
  - Scores are computed transposed, [t, q], so the A@V matmul needs no
    transposes anywhere: lhsT = K[d, t] slice, rhs = Q[d, q] slice.
  - exp runs on ScalarE directly from PSUM into SBUF (the PSUM->SBUF
    move is fused into the activation); softmax max-subtraction is
    skipped (scores ~ N(0,1), no overflow risk in f32).
  - The softmax denominator comes free from the A@V matmul via a ones
    column appended to V^T (lhsT is [128, 33]; row 32 of the psum
    accumulates sum_t exp(s)).
  - Normalization: DVE reciprocal of the denom row (lane 32), bounce
    through a DRAM scratch row, DMA-broadcast it across 32 partitions
    (stride-0 partition APs are only legal from DRAM), DVE multiply
    (one PSUM operand only), then a DMA moves the result to its head
    slot in attnout (DVE cannot shift partitions; DMA can).
  - This container's walrus allows ONE sem wait per instruction; a
    post-scheduling pass splits multi-wait instructions into
    single-wait same-engine nops (see _TileContextP).
"""

import numpy as np

import concourse.bass as bass
import concourse.mybir as mybir
import concourse.tile as tile
from concourse.vector_clock import ScopedClock
from concourse.bass_utils import run_bass_kernel_spmd

B, C, HH, WW = 4, 256, 48, 48
S = HH * WW            # 2304
NH, HD = 8, 32
SCALE = HD ** -0.5
SQ = S // 2            # 1152 queries per core
QC = 384               # q-chunk (fits one PSUM bank in f32)
NQC = SQ // QC         # 3
NTT = S // 128         # 18 t-tiles
TG = 3                 # t-tiles per exp group
NTG = NTT // TG        # 6
CT = C // 128          # 2 channel tiles
QCB = 192              # attention q-chunk (pairs: 6 subtiles of 256 = 3 banks)
NJB = SQ // QCB        # 6

F32 = mybir.dt.float32
F32R = mybir.dt.float32r
BF16 = mybir.dt.bfloat16
AF = mybir.ActivationFunctionType
ALU = mybir.AluOpType

N_CORES = 8


class _TileContextP(tile.TileContext):
    """TileContext adapted to a walrus that allows 1 sem wait/instruction.

    After Tile scheduling, every instruction carrying N>1 sem waits is
    rewritten to keep its last wait; the other N-1 waits move onto
    fresh single-wait nops inserted just before it on the same engine
    (engines execute their stream in order, so blocking at the nop is
    equivalent). The kernel-tail drain is built the same way.
    """

    def _split_multi_waits(self):
        nc = self.nc
        for fn in nc.m.functions:
            for bb in fn.blocks:
                new_insts = []
                for inst in bb.instructions:
                    si = inst.sync_info
                    if si is not None and len(si.on_wait) > 1:
                        waits = list(si.on_wait)
                        for w in waits[:-1]:
                            nop = mybir.InstNoOp(
                                name=nc.get_next_instruction_name(),
                                engine=inst.engine,
                                ins=[], outs=[],
                                sync_info=mybir.SyncInfo(on_wait=[w], on_update=[]),
                                bass_nofuse=True,
                            )
                            nc.register_instruction(nop, overwrite=True)
                            new_insts.append(nop)
                        inst.sync_info = mybir.SyncInfo(
                            on_wait=[waits[-1]], on_update=list(si.on_update)
                        )
                    new_insts.append(inst)
                bb.instructions = new_insts

    def _drain_and_barrier(self, tick_clock, wait_clock):
        carrier = self.nc.sync.nop(nofuse=True)
        wait_clock.add_sem_waits(
            carrier.ins, ScopedClock({None: tick_clock.global_clock})
        )
        self.nc.sync.drain()
        self.nc.all_engine_barrier()
        assert self.sems is not None
        popped = self.nc._tile_sem_poison_stack.pop()
        assert popped is self._sem_poison
        self.nc.clear_and_free_semaphores(list(self.sems.allocated().values()))
        self.nc.all_engine_barrier()
        self._split_multi_waits()


def _build_nc():
    nc = bass.Bass()

    xf_d = nc.dram_tensor("xf", [C, S], F32, kind="ExternalInput")
    xq_d = nc.dram_tensor("xq", [C, SQ], F32, kind="ExternalInput")
    wqt_d = nc.dram_tensor("wqt", [C, C], F32, kind="ExternalInput")
    wkt_d = nc.dram_tensor("wkt", [C, C], F32, kind="ExternalInput")
    wvt_d = nc.dram_tensor("wvt", [C, C], F32, kind="ExternalInput")
    wot_d = nc.dram_tensor("wot", [C, C], F32, kind="ExternalInput")
    bqp_d = nc.dram_tensor("bqp", [128, CT], F32, kind="ExternalInput")
    bkp_d = nc.dram_tensor("bkp", [128, CT], F32, kind="ExternalInput")
    bop_d = nc.dram_tensor("bop", [128, CT], F32, kind="ExternalInput")
    bv_d = nc.dram_tensor("bv", [C], F32, kind="ExternalInput")
    out_d = nc.dram_tensor("out", [C, SQ], F32, kind="ExternalOutput")

    with _TileContextP(nc) as tc:
        with (
            tc.tile_pool(name="singles", bufs=1) as singles,
            tc.tile_pool(name="sbig", bufs=1) as sbig,
            tc.tile_pool(name="expsp", bufs=3) as expsp,
            tc.tile_pool(name="smallp", bufs=4) as smallp,
            tc.tile_pool(name="outp", bufs=3) as outp,
            tc.tile_pool(name="drp", bufs=4, space="DRAM") as drp,
        ):
            # ---- static loads + f32r rounding copies --------------------
            w_ld = {}
            w_rb = {}
            for nm, d in (("wqt", wqt_d), ("wkt", wkt_d), ("wvt", wvt_d),
                          ("wot", wot_d)):
                ld = singles.tile([128, CT, C], F32, tag=f"{nm}_ld")
                nc.sync.dma_start(out=ld, in_=d.rearrange("(t p) o -> p t o", p=128))
                rb = singles.tile([128, CT, C], F32R, tag=f"{nm}_rb")
                nc.vector.tensor_copy(out=rb, in_=ld)
                w_ld[nm] = ld
                w_rb[nm] = rb
            wqt_sb, wkt_sb, wvt_sb, wot_sb = (
                w_rb["wqt"], w_rb["wkt"], w_rb["wvt"], w_rb["wot"])

            bqp_sb = singles.tile([128, CT], F32)
            bkp_sb = singles.tile([128, CT], F32)
            bop_sb = singles.tile([128, CT], F32)
            nc.sync.dma_start(out=bqp_sb, in_=bqp_d[:, :])
            nc.sync.dma_start(out=bkp_sb, in_=bkp_d[:, :])
            nc.sync.dma_start(out=bop_sb, in_=bop_d[:, :])

            bv_sb = singles.tile([128, C], F32)
            bv_ap = bv_d[:]
            nc.gpsimd.dma_start(
                out=bv_sb,
                in_=bass.AP(
                    tensor=bv_ap.tensor, offset=bv_ap.offset,
                    ap=[[0, 128]] + [list(a) for a in bv_ap.ap],
                ),
            )

            x_ld = sbig.tile([128, CT, S], F32)
            nc.sync.dma_start(out=x_ld, in_=xf_d.rearrange("(t p) s -> p t s", p=128))
            x_rb = sbig.tile([128, CT, S], F32R)
            nc.vector.tensor_copy(out=x_rb, in_=x_ld)
            xq_ld = sbig.tile([128, CT, SQ], F32)
            nc.sync.dma_start(out=xq_ld, in_=xq_d.rearrange("(t p) s -> p t s", p=128))
            xq_rb = sbig.tile([128, CT, SQ], F32R)
            nc.vector.tensor_copy(out=xq_rb, in_=xq_ld)

            k_sb = sbig.tile([128, CT, S], BF16)
            q_sb = sbig.tile([128, CT, SQ], BF16)
            vt_sb = sbig.tile([128, NTT, NH, HD + 1], BF16)
            att_sb = sbig.tile([128, CT, SQ], F32R)

            ones_f32 = singles.tile([128, NTT, NH], F32)
            nc.vector.memset(ones_f32, 1.0)
            nc.vector.tensor_copy(out=vt_sb[:, :, :, HD], in_=ones_f32)

            # ---- phase A: projections -----------------------------------
            with tc.tile_pool(name="psA", bufs=4, space="PSUM") as psA:
                # Q = WqT.T @ xq  (+bq), laid out [c, q]
                for ot in range(CT):
                    for j in range(NQC):
                        ps = psA.tile([128, QC], F32, tag="proj")
                        for kt in range(CT):
                            nc.tensor.matmul(
                                ps,
                                lhsT=wqt_sb[:, kt, ot * 128:(ot + 1) * 128],
                                rhs=xq_rb[:, kt, j * QC:(j + 1) * QC],
                                start=(kt == 0), stop=(kt == CT - 1),
                            )
                        nc.vector.tensor_scalar(
                            out=q_sb[:, ot, j * QC:(j + 1) * QC],
                            in0=ps, scalar1=bqp_sb[:, ot:ot + 1], scalar2=None,
                            op0=ALU.add,
                        )
                # K = WkT.T @ xf  (+bk), laid out [c, t]
                for ot in range(CT):
                    for j in range(S // QC):
                        ps = psA.tile([128, QC], F32, tag="proj")
                        for kt in range(CT):
                            nc.tensor.matmul(
                                ps,
                                lhsT=wkt_sb[:, kt, ot * 128:(ot + 1) * 128],
                                rhs=x_rb[:, kt, j * QC:(j + 1) * QC],
                                start=(kt == 0), stop=(kt == CT - 1),
                            )
                        nc.vector.tensor_scalar(
                            out=k_sb[:, ot, j * QC:(j + 1) * QC],
                            in0=ps, scalar1=bkp_sb[:, ot:ot + 1], scalar2=None,
                            op0=ALU.add,
                        )
                # V^T = (x slice).T @ WvT  (+bv), laid out [t, c] per tile
                for st in range(NTT):
                    ps = psA.tile([128, C], F32, tag="projv")
                    for kt in range(CT):
                        nc.tensor.matmul(
                            ps,
                            lhsT=x_rb[:, kt, st * 128:(st + 1) * 128],
                            rhs=wvt_sb[:, kt, :],
                            start=(kt == 0), stop=(kt == CT - 1),
                        )
                    nc.vector.tensor_tensor(
                        out=vt_sb[:, st, :, 0:HD],
                        in0=ps.rearrange("p (h d) -> p h d", d=HD),
                        in1=bv_sb.rearrange("p (h d) -> p h d", d=HD),
                        op=ALU.add,
                    )

            # ---- phase B: attention -------------------------------------
            # Heads run in pairs (2i, 2i+1). Both heads' scores for a
            # 3-t-tile group live in ONE psum tensor [128, 6, 256] (3
            # banks), so a single exp instruction (N=1152) covers the
            # pair and both heads' next scores become ready together --
            # the PE then executes them as concurrent row-group pairs.
            # Scores/A@V run in bf16 (full rate at any N; fp32r would
            # drop to 1/4 rate below N=256). A@V packs the pair into
            # column strips {0,64} of one bank.
            with (
                tc.tile_pool(name="scp", bufs=2, space="PSUM") as scp,
                tc.tile_pool(name="avp", bufs=2, space="PSUM") as avp,
            ):
                for hp in range(NH // 2):
                    ha, hb = 2 * hp, 2 * hp + 1
                    heads = ((ha // 4, 32 * (ha % 4)), (hb // 4, 32 * (hb % 4)))
                    for j in range(NJB):
                        js = slice(j * QCB, (j + 1) * QCB)
                        av = avp.tile([97, QCB], F32, tag="av")
                        for g in range(NTG):
                            sc = scp.tile([128, 2 * TG, 256], F32, tag="sc")
                            for tt in range(TG):
                                t0 = (g * TG + tt) * 128
                                for hi, (ct, co) in enumerate(heads):
                                    nc.tensor.matmul(
                                        sc[:, hi * TG + tt, 0:QCB],
                                        lhsT=k_sb[co:co + HD, ct, t0:t0 + 128],
                                        rhs=q_sb[co:co + HD, ct, js],
                                        start=True, stop=True,
                                        tile_position=(co, 0),
                                    )
                            ex = expsp.tile([128, 2 * TG, QCB], BF16, tag="ex")
                            nc.scalar.activation(
                                out=ex, in_=sc[:, :, 0:QCB], func=AF.Exp, scale=SCALE,
                            )
                            for tt in range(TG):
                                st = g * TG + tt
                                first = (g == 0 and tt == 0)
                                last = (g == NTG - 1 and tt == TG - 1)
                                for hi, (ct, co) in enumerate(heads):
                                    nc.tensor.matmul(
                                        av[64 * hi:64 * hi + HD + 1, :],
                                        lhsT=vt_sb[:, st, 2 * hp + hi, :],
                                        rhs=ex[:, hi * TG + tt, :],
                                        start=first, stop=last,
                                        tile_position=(0, 64 * hi),
                                        skip_group_check=True,
                                    )
                        # normalize both heads: reciprocal of the denom
                        # lanes (32, 96), DRAM-bounce broadcast per head,
                        # lane-aligned multiplies, DMA shift into att_sb.
                        rec = smallp.tile([97, QCB], F32, tag="rec")
                        nc.vector.reciprocal(rec[HD:HD + 1, :], av[HD:HD + 1, :])
                        nc.vector.reciprocal(rec[96:97, :], av[96:97, :])
                        dscr = drp.tile([2, QCB], F32, tag="dscr")
                        nc.sync.dma_start(out=dscr[0:1, :], in_=rec[HD:HD + 1, :])
                        nc.sync.dma_start(out=dscr[1:2, :], in_=rec[96:97, :])
                        bc = smallp.tile([96, QCB], F32, tag="bc")
                        for hi in range(2):
                            dap = dscr[hi:hi + 1, :]
                            nc.gpsimd.dma_start(
                                out=bc[64 * hi:64 * hi + HD, :],
                                in_=bass.AP(
                                    tensor=dap.tensor, offset=dap.offset,
                                    ap=[[0, HD]] + [list(a) for a in dap.ap[1:]],
                                ),
                            )
                        nrm = smallp.tile([96, QCB], F32R, tag="nrm")
                        for hi, (ct, co) in enumerate(heads):
                            nc.vector.tensor_tensor(
                                out=nrm[64 * hi:64 * hi + HD, :],
                                in0=av[64 * hi:64 * hi + HD, :],
                                in1=bc[64 * hi:64 * hi + HD, :], op=ALU.mult,
                            )
                            nc.sync.dma_start(
                                out=att_sb[co:co + HD, ct, js],
                                in_=nrm[64 * hi:64 * hi + HD, :],
                            )

            # ---- phase C: output projection + bias + residual -----------
            out_r = out_d.rearrange("(t p) q -> p t q", p=128)
            with tc.tile_pool(name="psC", bufs=2, space="PSUM") as psC:
                for ot in range(CT):
                    for j in range(NQC):
                        ps = psC.tile([128, QC], F32, tag="proj")
                        for kt in range(CT):
                            nc.tensor.matmul(
                                ps,
                                lhsT=wot_sb[:, kt, ot * 128:(ot + 1) * 128],
                                rhs=att_sb[:, kt, j * QC:(j + 1) * QC],
                                start=(kt == 0), stop=(kt == CT - 1),
                            )
                        ob = outp.tile([128, QC], F32, tag="ob")
                        nc.vector.tensor_scalar(
                            out=ob, in0=ps, scalar1=bop_sb[:, ot:ot + 1],
                            scalar2=None, op0=ALU.add,
                        )
                        nc.vector.tensor_tensor(
                            out=ob, in0=ob,
                            in1=xq_ld[:, ot, j * QC:(j + 1) * QC], op=ALU.add,
                        )
                        nc.sync.dma_start(
                            out=out_r[:, ot, j * QC:(j + 1) * QC], in_=ob,
                        )

    return nc


_NC = None
LAST_RESULTS = None
TRACE = False


def _get_nc():
    global _NC
    if _NC is None:
        _NC = _build_nc()
    return _NC


def kernel(x, Wq, bq, Wk, bk, Wv, bv, Wo, bo):
    global LAST_RESULTS
    x = np.ascontiguousarray(np.asarray(x, dtype=np.float32).reshape(B, C, S))
    wqt = np.ascontiguousarray(np.asarray(Wq, dtype=np.float32).T)
    wkt = np.ascontiguousarray(np.asarray(Wk, dtype=np.float32).T)
    wvt = np.ascontiguousarray(np.asarray(Wv, dtype=np.float32).T)
    wot = np.ascontiguousarray(np.asarray(Wo, dtype=np.float32).T)
    bqp = np.ascontiguousarray(np.asarray(bq, dtype=np.float32).reshape(CT, 128).T)
    bkp = np.ascontiguousarray(np.asarray(bk, dtype=np.float32).reshape(CT, 128).T)
    bop = np.ascontiguousarray(np.asarray(bo, dtype=np.float32).reshape(CT, 128).T)
    bvv = np.ascontiguousarray(np.asarray(bv, dtype=np.float32))

    in_maps = []
    for core in range(N_CORES):
        b, half = divmod(core, 2)
        qlo = half * SQ
        in_maps.append({
            "xf": x[b],
            "xq": np.ascontiguousarray(x[b][:, qlo:qlo + SQ]),
            "wqt": wqt, "wkt": wkt, "wvt": wvt, "wot": wot,
            "bqp": bqp, "bkp": bkp, "bop": bop, "bv": bvv,
        })

    res = run_bass_kernel_spmd(_get_nc(), in_maps, list(range(N_CORES)), trace=TRACE)
    LAST_RESULTS = res

    out = np.empty((B, C, S), dtype=np.float32)
    for core in range(N_CORES):
        b, half = divmod(core, 2)
        qlo = half * SQ
        out[b][:, qlo:qlo + SQ] = res.results[core]["out"]
    return out.reshape(B, C, HH, WW)


# revision 11
# speedup vs baseline: 1.5765x; 1.1974x over previous
"""Multi-head self-attention (B=4, C=256, H=W=48, NH=8) on 8 TRN2 NeuronCores.

Sharding: 8 shards = 4 batches x 2 query-halves (no collectives).
Each core: K,V projections for its batch over all S=2304 positions,
Q projection for its 1152-query half, attention for all 8 heads over
its query half, output projection + bias + residual for its disjoint
[256, 1152] output slice.

Kernel layout notes:
  - All matmuls run as float32r (full PE rate at free-dim >= 256). The
    BIR verifier requires f32r matmul inputs to be *produced rounded*
    by a compute op, so Q/K/V^T/exp tiles are written as f32r by their
    producing DVE/ACT ops, and x / weights get explicit DVE rounding
    copies after the DMA load.
  - Scores are computed transposed, [t, q], so the A@V matmul needs no
    transposes anywhere: lhsT = K[d, t] slice, rhs = Q[d, q] slice.
  - exp runs on ScalarE directly from PSUM into SBUF (the PSUM->SBUF
    move is fused into the activation); softmax max-subtraction is
    skipped (scores ~ N(0,1), no overflow risk in f32).
  - The softmax denominator comes free from the A@V matmul via a ones
    column appended to V^T (lhsT is [128, 33]; row 32 of the psum
    accumulates sum_t exp(s)).
  - Normalization: DVE reciprocal of the denom row (lane 32), bounce
    through a DRAM scratch row, DMA-broadcast it across 32 partitions
    (stride-0 partition APs are only legal from DRAM), DVE multiply
    (one PSUM operand only), then a DMA moves the result to its head
    slot in attnout (DVE cannot shift partitions; DMA can).
  - This container's walrus allows ONE sem wait per instruction; a
    post-scheduling pass splits multi-wait instructions into
    single-wait same-engine nops (see _TileContextP).
"""

import numpy as np

import concourse.bass as bass
import concourse.mybir as mybir
import concourse.tile as tile
from concourse.vector_clock import ScopedClock
from concourse.bass_utils import run_bass_kernel_spmd

B, C, HH, WW = 4, 256, 48, 48
S = HH * WW            # 2304
NH, HD = 8, 32
SCALE = HD ** -0.5
SQ = S // 2            # 1152 queries per core
QC = 384               # q-chunk (fits one PSUM bank in f32)
NQC = SQ // QC         # 3
NTT = S // 128         # 18 t-tiles
TG = 3                 # t-tiles per exp group
NTG = NTT // TG        # 6
CT = C // 128          # 2 channel tiles
QCB = 192              # attention q-chunk (pairs: 6 subtiles of 256 = 3 banks)
NJB = SQ // QCB        # 6

F32 = mybir.dt.float32
F32R = mybir.dt.float32r
BF16 = mybir.dt.bfloat16
AF = mybir.ActivationFunctionType
ALU = mybir.AluOpType

N_CORES = 8


class _TileContextP(tile.TileContext):
    """TileContext adapted to a walrus that allows 1 sem wait/instruction.

    After Tile scheduling, every instruction carrying N>1 sem waits is
    rewritten to keep its last wait; the other N-1 waits move onto
    fresh single-wait nops inserted just before it on the same engine
    (engines execute their stream in order, so blocking at the nop is
    equivalent). The kernel-tail drain is built the same way.
    """

    def _split_multi_waits(self):
        nc = self.nc
        for fn in nc.m.functions:
            for bb in fn.blocks:
                new_insts = []
                for inst in bb.instructions:
                    si = inst.sync_info
                    if si is not None and len(si.on_wait) > 1:
                        waits = list(si.on_wait)
                        for w in waits[:-1]:
                            nop = mybir.InstNoOp(
                                name=nc.get_next_instruction_name(),
                                engine=inst.engine,
                                ins=[], outs=[],
                                sync_info=mybir.SyncInfo(on_wait=[w], on_update=[]),
                                bass_nofuse=True,
                            )
                            nc.register_instruction(nop, overwrite=True)
                            new_insts.append(nop)
                        inst.sync_info = mybir.SyncInfo(
                            on_wait=[waits[-1]], on_update=list(si.on_update)
                        )
                    new_insts.append(inst)
                bb.instructions = new_insts

    def _drain_and_barrier(self, tick_clock, wait_clock):
        carrier = self.nc.sync.nop(nofuse=True)
        wait_clock.add_sem_waits(
            carrier.ins, ScopedClock({None: tick_clock.global_clock})
        )
        self.nc.sync.drain()
        self.nc.all_engine_barrier()
        assert self.sems is not None
        popped = self.nc._tile_sem_poison_stack.pop()
        assert popped is self._sem_poison
        self.nc.clear_and_free_semaphores(list(self.sems.allocated().values()))
        self.nc.all_engine_barrier()
        self._split_multi_waits()


def _build_nc():
    nc = bass.Bass()

    xf_d = nc.dram_tensor("xf", [C, S], F32, kind="ExternalInput")
    xq_d = nc.dram_tensor("xq", [C, SQ], F32, kind="ExternalInput")
    wqt_d = nc.dram_tensor("wqt", [C, C], F32, kind="ExternalInput")
    wkt_d = nc.dram_tensor("wkt", [C, C], F32, kind="ExternalInput")
    wvt_d = nc.dram_tensor("wvt", [C, C], F32, kind="ExternalInput")
    wot_d = nc.dram_tensor("wot", [C, C], F32, kind="ExternalInput")
    bqp_d = nc.dram_tensor("bqp", [128, CT], F32, kind="ExternalInput")
    bkp_d = nc.dram_tensor("bkp", [128, CT], F32, kind="ExternalInput")
    bop_d = nc.dram_tensor("bop", [128, CT], F32, kind="ExternalInput")
    bv_d = nc.dram_tensor("bv", [C], F32, kind="ExternalInput")
    out_d = nc.dram_tensor("out", [C, SQ], F32, kind="ExternalOutput")

    # attention q-chunks: 4x256 + 1x128 (bf16 matmuls run 1 cyc/row at any N)
    JCH = [(0, 256), (256, 256), (512, 256), (768, 256), (1024, 128)]

    with _TileContextP(nc) as tc:
        with (
            tc.tile_pool(name="singles", bufs=1) as singles,
            tc.tile_pool(name="sbig", bufs=1) as sbig,
            tc.tile_pool(name="expsp", bufs=3) as expsp,
            tc.tile_pool(name="smallp", bufs=4) as smallp,
            tc.tile_pool(name="outp", bufs=3) as outp,
            tc.tile_pool(name="drp", bufs=4, space="DRAM") as drp,
        ):
            # ---- static loads + rounding/cast copies --------------------
            # QKV projections run in bf16 (their outputs are bf16-rounded
            # anyway); the output projection stays fp32r for precision.
            w_bf = {}
            for nm, d in (("wqt", wqt_d), ("wkt", wkt_d), ("wvt", wvt_d)):
                ld = singles.tile([128, CT, C], F32, tag=f"{nm}_ld")
                nc.sync.dma_start(out=ld, in_=d.rearrange("(t p) o -> p t o", p=128))
                rb = singles.tile([128, CT, C], BF16, tag=f"{nm}_bf")
                nc.vector.tensor_copy(out=rb, in_=ld)
                w_bf[nm] = rb
            wqt_sb, wkt_sb, wvt_sb = w_bf["wqt"], w_bf["wkt"], w_bf["wvt"]
            wot_ld = singles.tile([128, CT, C], F32, tag="wot_ld")
            nc.sync.dma_start(out=wot_ld, in_=wot_d.rearrange("(t p) o -> p t o", p=128))
            wot_sb = singles.tile([128, CT, C], F32R, tag="wot_rb")
            nc.vector.tensor_copy(out=wot_sb, in_=wot_ld)

            bqp_sb = singles.tile([128, CT], F32)
            bkp_sb = singles.tile([128, CT], F32)
            bop_sb = singles.tile([128, CT], F32)
            nc.sync.dma_start(out=bqp_sb, in_=bqp_d[:, :])
            nc.sync.dma_start(out=bkp_sb, in_=bkp_d[:, :])
            nc.sync.dma_start(out=bop_sb, in_=bop_d[:, :])

            bv_sb = singles.tile([128, C], F32)
            bv_ap = bv_d[:]
            nc.gpsimd.dma_start(
                out=bv_sb,
                in_=bass.AP(
                    tensor=bv_ap.tensor, offset=bv_ap.offset,
                    ap=[[0, 128]] + [list(a) for a in bv_ap.ap],
                ),
            )

            x_ld = sbig.tile([128, CT, S], F32)
            nc.sync.dma_start(out=x_ld, in_=xf_d.rearrange("(t p) s -> p t s", p=128))
            x_bf = sbig.tile([128, CT, S], BF16)
            nc.vector.tensor_copy(out=x_bf, in_=x_ld)
            xq_ld = sbig.tile([128, CT, SQ], F32)
            nc.sync.dma_start(out=xq_ld, in_=xq_d.rearrange("(t p) s -> p t s", p=128))
            xq_bf = sbig.tile([128, CT, SQ], BF16)
            nc.vector.tensor_copy(out=xq_bf, in_=xq_ld)

            k_sb = sbig.tile([128, CT, S], BF16)
            q_sb = sbig.tile([128, CT, SQ], BF16)
            vt_sb = sbig.tile([128, NTT, NH, HD + 1], BF16)
            att_sb = sbig.tile([128, CT, SQ], F32R)

            ones_f32 = singles.tile([128, NTT, NH], F32)
            nc.vector.memset(ones_f32, 1.0)
            nc.vector.tensor_copy(out=vt_sb[:, :, :, HD], in_=ones_f32)

            # ---- phase A: projections (bf16) ----------------------------
            with tc.tile_pool(name="psA", bufs=4, space="PSUM") as psA:
                for ot in range(CT):
                    for j in range(NQC):
                        ps = psA.tile([128, QC], F32, tag="proj")
                        for kt in range(CT):
                            nc.tensor.matmul(
                                ps,
                                lhsT=wqt_sb[:, kt, ot * 128:(ot + 1) * 128],
                                rhs=xq_bf[:, kt, j * QC:(j + 1) * QC],
                                start=(kt == 0), stop=(kt == CT - 1),
                            )
                        nc.vector.tensor_scalar(
                            out=q_sb[:, ot, j * QC:(j + 1) * QC],
                            in0=ps, scalar1=bqp_sb[:, ot:ot + 1], scalar2=None,
                            op0=ALU.add,
                        )
                for ot in range(CT):
                    for j in range(S // QC):
                        ps = psA.tile([128, QC], F32, tag="proj")
                        for kt in range(CT):
                            nc.tensor.matmul(
                                ps,
                                lhsT=wkt_sb[:, kt, ot * 128:(ot + 1) * 128],
                                rhs=x_bf[:, kt, j * QC:(j + 1) * QC],
                                start=(kt == 0), stop=(kt == CT - 1),
                            )
                        nc.vector.tensor_scalar(
                            out=k_sb[:, ot, j * QC:(j + 1) * QC],
                            in0=ps, scalar1=bkp_sb[:, ot:ot + 1], scalar2=None,
                            op0=ALU.add,
                        )
                for st in range(NTT):
                    ps = psA.tile([128, C], F32, tag="projv")
                    for kt in range(CT):
                        nc.tensor.matmul(
                            ps,
                            lhsT=x_bf[:, kt, st * 128:(st + 1) * 128],
                            rhs=wvt_sb[:, kt, :],
                            start=(kt == 0), stop=(kt == CT - 1),
                        )
                    nc.vector.tensor_tensor(
                        out=vt_sb[:, st, :, 0:HD],
                        in0=ps.rearrange("p (h d) -> p h d", d=HD),
                        in1=bv_sb.rearrange("p (h d) -> p h d", d=HD),
                        op=ALU.add,
                    )

            # ---- phase B: attention -------------------------------------
            # Heads run in pairs (2i, 2i+1). Both heads' scores for a
            # 3-t-tile group live in ONE contiguous psum tensor
            # [128, 6, 256] (3 banks), so a single exp instruction
            # (N=1536) covers the pair and both heads' next scores
            # become ready together -- the PE executes them as
            # concurrent row-group pairs. A@V packs the pair into
            # column strips {0,64} of one bank and yields the softmax
            # denominators via the ones column of V^T.
            with (
                tc.tile_pool(name="scp", bufs=2, space="PSUM") as scp,
                tc.tile_pool(name="avp", bufs=2, space="PSUM") as avp,
            ):
                for hp in range(NH // 2):
                    ha, hb = 2 * hp, 2 * hp + 1
                    heads = ((ha // 4, 32 * (ha % 4)), (hb // 4, 32 * (hb % 4)))
                    for j0, ln in JCH:
                        js = slice(j0, j0 + ln)
                        av = avp.tile([97, 256], F32, tag="av")
                        for g in range(NTG):
                            sc = scp.tile([128, 2 * TG, 256], F32, tag="sc")
                            for tt in range(TG):
                                t0 = (g * TG + tt) * 128
                                for hi, (ct, co) in enumerate(heads):
                                    nc.tensor.matmul(
                                        sc[:, hi * TG + tt, 0:ln],
                                        lhsT=k_sb[co:co + HD, ct, t0:t0 + 128],
                                        rhs=q_sb[co:co + HD, ct, js],
                                        start=True, stop=True,
                                        tile_position=(co, 0),
                                    )
                            ex = expsp.tile([128, 2 * TG, 256], BF16, tag="ex")
                            nc.scalar.activation(
                                out=ex[:, :, 0:ln], in_=sc[:, :, 0:ln],
                                func=AF.Exp, scale=SCALE,
                            )
                            for tt in range(TG):
                                st = g * TG + tt
                                first = (g == 0 and tt == 0)
                                last = (g == NTG - 1 and tt == TG - 1)
                                for hi, (ct, co) in enumerate(heads):
                                    nc.tensor.matmul(
                                        av[64 * hi:64 * hi + HD + 1, 0:ln],
                                        lhsT=vt_sb[:, st, 2 * hp + hi, :],
                                        rhs=ex[:, hi * TG + tt, 0:ln],
                                        start=first, stop=last,
                                        tile_position=(0, 64 * hi),
                                        skip_group_check=True,
                                    )
                        # normalize: reciprocal of the denom lanes (32, 96),
                        # DRAM-bounce broadcast, lane-aligned multiplies,
                        # DMA shift into att_sb head slots.
                        rec = smallp.tile([97, 256], F32, tag="rec")
                        nc.vector.reciprocal(rec[HD:HD + 1, 0:ln], av[HD:HD + 1, 0:ln])
                        nc.vector.reciprocal(rec[96:97, 0:ln], av[96:97, 0:ln])
                        dscr = drp.tile([2, 256], F32, tag="dscr")
                        nc.sync.dma_start(out=dscr[0:1, 0:ln], in_=rec[HD:HD + 1, 0:ln])
                        nc.sync.dma_start(out=dscr[1:2, 0:ln], in_=rec[96:97, 0:ln])
                        bc = smallp.tile([96, 256], F32, tag="bc")
                        for hi in range(2):
                            dap = dscr[hi:hi + 1, 0:ln]
                            nc.gpsimd.dma_start(
                                out=bc[64 * hi:64 * hi + HD, 0:ln],
                                in_=bass.AP(
                                    tensor=dap.tensor, offset=dap.offset,
                                    ap=[[0, HD]] + [list(a) for a in dap.ap[1:]],
                                ),
                            )
                        nrm = smallp.tile([96, 256], F32R, tag="nrm")
                        for hi, (ct, co) in enumerate(heads):
                            nc.vector.tensor_tensor(
                                out=nrm[64 * hi:64 * hi + HD, 0:ln],
                                in0=av[64 * hi:64 * hi + HD, 0:ln],
                                in1=bc[64 * hi:64 * hi + HD, 0:ln], op=ALU.mult,
                            )
                            nc.sync.dma_start(
                                out=att_sb[co:co + HD, ct, js],
                                in_=nrm[64 * hi:64 * hi + HD, 0:ln],
                            )

            # ---- phase C: output projection + bias + residual (fp32r) ---
            out_r = out_d.rearrange("(t p) q -> p t q", p=128)
            with tc.tile_pool(name="psC", bufs=2, space="PSUM") as psC:
                for ot in range(CT):
                    for j in range(NQC):
                        ps = psC.tile([128, QC], F32, tag="proj")
                        for kt in range(CT):
                            nc.tensor.matmul(
                                ps,
                                lhsT=wot_sb[:, kt, ot * 128:(ot + 1) * 128],
                                rhs=att_sb[:, kt, j * QC:(j + 1) * QC],
                                start=(kt == 0), stop=(kt == CT - 1),
                            )
                        ob = outp.tile([128, QC], F32, tag="ob")
                        nc.vector.tensor_scalar(
                            out=ob, in0=ps, scalar1=bop_sb[:, ot:ot + 1],
                            scalar2=None, op0=ALU.add,
                        )
                        nc.vector.tensor_tensor(
                            out=ob, in0=ob,
                            in1=xq_ld[:, ot, j * QC:(j + 1) * QC], op=ALU.add,
                        )
                        nc.sync.dma_start(
                            out=out_r[:, ot, j * QC:(j + 1) * QC], in_=ob,
                        )

    return nc


_NC = None
LAST_RESULTS = None
TRACE = False


def _get_nc():
    global _NC
    if _NC is None:
        _NC = _build_nc()
    return _NC


def kernel(x, Wq, bq, Wk, bk, Wv, bv, Wo, bo):
    global LAST_RESULTS
    x = np.ascontiguousarray(np.asarray(x, dtype=np.float32).reshape(B, C, S))
    wqt = np.ascontiguousarray(np.asarray(Wq, dtype=np.float32).T)
    wkt = np.ascontiguousarray(np.asarray(Wk, dtype=np.float32).T)
    wvt = np.ascontiguousarray(np.asarray(Wv, dtype=np.float32).T)
    wot = np.ascontiguousarray(np.asarray(Wo, dtype=np.float32).T)
    bqp = np.ascontiguousarray(np.asarray(bq, dtype=np.float32).reshape(CT, 128).T)
    bkp = np.ascontiguousarray(np.asarray(bk, dtype=np.float32).reshape(CT, 128).T)
    bop = np.ascontiguousarray(np.asarray(bo, dtype=np.float32).reshape(CT, 128).T)
    bvv = np.ascontiguousarray(np.asarray(bv, dtype=np.float32))

    in_maps = []
    for core in range(N_CORES):
        b, half = divmod(core, 2)
        qlo = half * SQ
        in_maps.append({
            "xf": x[b],
            "xq": np.ascontiguousarray(x[b][:, qlo:qlo + SQ]),
            "wqt": wqt, "wkt": wkt, "wvt": wvt, "wot": wot,
            "bqp": bqp, "bkp": bkp, "bop": bop, "bv": bvv,
        })

    res = run_bass_kernel_spmd(_get_nc(), in_maps, list(range(N_CORES)), trace=TRACE)
    LAST_RESULTS = res

    out = np.empty((B, C, S), dtype=np.float32)
    for core in range(N_CORES):
        b, half = divmod(core, 2)
        qlo = half * SQ
        out[b][:, qlo:qlo + SQ] = res.results[core]["out"]
    return out.reshape(B, C, HH, WW)
